# revision 21
# baseline (speedup 1.0000x reference)
"""Multi-head causal attention (B=4, S=2048, D=1024, H=16, hd=64) on 8 TRN2 cores.

Sharding: core c -> (batch b = c//2, head-group hg = c%2 of 8 heads).
Each core computes its batch's QKV projection for its 8 heads (tensor-parallel
column split of Wq/Wk/Wv), causal attention, and a partial output projection
(row-parallel split of Wo). Host sums the two partials per batch and adds bias.

Device-side layout avoids all transposes:
  - host passes x[b] transposed (xT [1024, 2048])
  - Q^T/K^T computed as [d, seq] via lhsT=W tile, rhs=xT
  - V computed natural [seq, d] via lhsT=xT tile, rhs=W, stored with a ones
    column per head (v_aug, M=65) so the PV matmul also accumulates the
    softmax denominator
  - scores computed as S^T [keys, q]; exp on ACT (scale=1/8); causal handling:
    fully-masked key tiles skipped, fully-masked columns of diagonal tiles
    never computed (column-restricted matmul/exp), only the 128-wide diagonal
    window gets a DVE mask multiply
  - 1/denom broadcast across partitions via a K=1 matmul, normalize on DVE
    writing straight into ctxT [feat, q] which is the lhsT of the out-proj
All matmuls in float32r (full PE rate at N>=256). Inputs are declared float32r
in DRAM so plain HWDGE DMAs feed the PE without cast copies.
"""
import os
import sys

import numpy as np
import ml_dtypes

try:
    import concourse  # noqa: F401
except ImportError:
    sys.path.insert(0, "/opt/trn_rl_repo")

import concourse.bass as bass  # noqa: F401  (bass must import before bacc)
import concourse.mybir as mybir
import concourse.tile as tile
from concourse import bacc
from concourse.bass_utils import run_bass_kernel_spmd

F32 = mybir.dt.float32
F32R = mybir.dt.float32r
BF16 = mybir.dt.bfloat16
EXP = mybir.ActivationFunctionType.Exp

B, S, DM = 4, 2048, 1024          # batch, seq, model dim
H, HD = 16, 64                    # total heads, head dim
HG = 8                            # heads per core (head group)
DG = HG * HD                      # 512 = feature dim per core
N = 512                           # matmul moving free dim
P = 128                           # partitions
NQT = S // N                      # 4 q-tiles of 512
NKT = S // P                      # 16 key tiles of 128
NMT = DM // P                     # 8 model-dim tiles

LOOKAHEAD = 2                     # score-matmul lookahead before PV matmuls

_cached = {}


def _build():
    nc = bacc.Bacc("TRN2", target_bir_lowering=False, debug=False)

    xT_d = nc.dram_tensor("xT", [DM, S], F32R, kind="ExternalInput").ap()
    wq_d = nc.dram_tensor("wq", [DM, DG], F32R, kind="ExternalInput").ap()
    wk_d = nc.dram_tensor("wk", [DM, DG], F32R, kind="ExternalInput").ap()
    wv_d = nc.dram_tensor("wv", [DM, DG], F32R, kind="ExternalInput").ap()
    wo_d = nc.dram_tensor("wo", [DG, DM], BF16, kind="ExternalInput").ap()
    out_d = nc.dram_tensor("out", [S, DM], F32, kind="ExternalOutput").ap()

    with tile.TileContext(nc) as tc, (
            nc.allow_low_precision(reason="fp32r matmul staging")), (
            tc.tile_pool(name="sb", bufs=1)) as sb, (
            tc.tile_pool(name="ps", bufs=1, space="PSUM")) as ps:

        # ---- static tiles ----
        kT = [sb.tile([P, S], F32R, name=f"kT{i}") for i in range(4)]
        # v_aug[j][:, s, h, :]: [8 si, 8 heads, 65] (64 V cols + ones col)
        v_aug2 = [sb.tile([P, 8, HG, HD + 1], F32R, name=f"vaug{i}")
                  for i in range(2)]
        v_aug = [v_aug2[i // 8][:, i % 8] for i in range(NKT)]
        tri = sb.tile([P, P], F32R, name="tri")  # tri[k,q] = 1 iff k <= q
        ones64 = sb.tile([1, HD], F32R, name="ones64")
        mask_f32 = sb.tile([P, P], F32, tag="ost", bufs=2, name="mask_f32")
        nc.gpsimd.memset(mask_f32[:], 1.0)
        nc.gpsimd.affine_select(
            out=tri[:], in_=mask_f32[:],
            compare_op=mybir.AluOpType.is_ge,
            fill=0.0, base=0,
            pattern=[[1, P]], channel_multiplier=-1,
        )  # keep where q - k >= 0
        ones_f32 = sb.tile([P, HD], F32, name="ones_f32")
        nc.gpsimd.memset(ones_f32[:], 1.0)
        nc.vector.tensor_copy(ones64[:], ones_f32[:1, :])

        # ---- input DMAs (rotating slots auto-pace the prefetch) ----
        def load_w(w_d, nm):
            wt = []
            for mi in range(NMT):
                w = sb.tile([P, DG], F32R, tag=f"{nm}{mi}", name=f"{nm}{mi}")
                nc.sync.dma_start(out=w[:], in_=w_d[mi * P:(mi + 1) * P, :])
                wt.append(w)
            return wt

        wvt = load_w(wv_d, "wv")
        xTt = [[None] * NQT for _ in range(NMT)]
        for qb in range(NQT):
            for mi in range(NMT):
                xt = sb.tile([P, N], F32R, tag=f"x{mi}", bufs=2,
                             name=f"x{mi}_{qb}")
                nc.sync.dma_start(
                    out=xt[:],
                    in_=xT_d[mi * P:(mi + 1) * P, qb * N:(qb + 1) * N])
                xTt[mi][qb] = xt
        wqt = load_w(wq_d, "wq")
        wkt = load_w(wk_d, "wk")
        wot = []
        for ft in range(4):
            w = sb.tile([P, DM], BF16, tag=f"wo{ft}", name=f"wo{ft}")
            nc.sync.dma_start(out=w[:], in_=wo_d[ft * P:(ft + 1) * P, :])
            wot.append(w)

        # rotating per-q-tile tiles
        qTq = [[None] * 4 for _ in range(NQT)]   # [qi][t] -> [128, 512]
        ctxq = [[None] * 4 for _ in range(NQT)]  # [qi][t] -> [128, 512]

        # ---- emission helpers ----
        def v_group(qi, j):
            si = 4 * qi + j
            p = ps.tile([P, N], F32, tag="big", bufs=2, name="psv")
            for mi in range(NMT):
                nc.tensor.matmul(
                    p[:], xTt[mi][qi][:, j * P:(j + 1) * P], wvt[mi][:],
                    start=(mi == 0), stop=(mi == NMT - 1),
                )
            nc.vector.tensor_copy(
                v_aug[si][:, :, :HD], p.rearrange("p (h d) -> p h d", d=HD))
            nc.vector.tensor_copy(v_aug[si][:, :, HD], ones_f32[:, :HG])

        def q_group(qi, dt):
            p = ps.tile([P, N], F32, tag="big", bufs=2, name="psq")
            for mi in range(NMT):
                nc.tensor.matmul(
                    p[:], wqt[mi][:, dt * P:(dt + 1) * P], xTt[mi][qi][:],
                    start=(mi == 0), stop=(mi == NMT - 1),
                )
            dst = sb.tile([P, N], F32R, tag=f"qT{dt}", bufs=2,
                          name=f"qT{dt}_{qi}")
            nc.vector.tensor_copy(dst[:], p[:])
            qTq[qi][dt] = dst

        def k_group(qi, dt):
            p = ps.tile([P, N], F32, tag="big", bufs=2, name="psk")
            for mi in range(NMT):
                nc.tensor.matmul(
                    p[:], wkt[mi][:, dt * P:(dt + 1) * P], xTt[mi][qi][:],
                    start=(mi == 0), stop=(mi == NMT - 1),
                )
            nc.vector.tensor_copy(kT[dt][:, qi * N:(qi + 1) * N], p[:])

        pending_fin = []

        def finalize():
            # deferred normalization: recip -> K=1 broadcast matmul -> DVE
            # multiply straight into ctxT (never blocks the PE stream)
            if not pending_fin:
                return
            fqi, fh, ctx_acc = pending_fin.pop()
            ft_, fhb = fh // 2, (fh % 2) * HD
            recip = sb.tile([1, N], F32R, tag="recip", bufs=1, name="recip")
            nc.vector.reciprocal(recip[:], ctx_acc[HD:HD + 1, :])
            bc = ps.tile([P, N], F32, tag="big", bufs=2, name="bc")
            nc.tensor.matmul(bc[:HD, :], ones64[:], recip[:],
                             start=True, stop=True)
            bc_sb = sb.tile([HD, N], F32R, tag="bcsb", bufs=1, name="bcsb")
            nc.vector.tensor_copy(bc_sb[:], bc[:HD, :])
            nc.vector.tensor_mul(
                ctxq[fqi][ft_][fhb:fhb + HD, :], ctx_acc[:HD, :], bc_sb[:])

        pending_op = []

        def emit_op_group():
            if pending_op:
                pending_op.pop(0)()

        def outproj_group(qi, s4, nt):
            s = qi * 4 + s4
            p = ps.tile([P, N], F32, tag="big", bufs=2, name="ou")
            for ft in range(4):
                nc.tensor.matmul(
                    p[:],
                    ctxq[qi][ft][:, s4 * P:(s4 + 1) * P],
                    wot[ft][:, nt * N:(nt + 1) * N],
                    start=(ft == 0), stop=(ft == 3),
                )
            ost = sb.tile([P, N], F32, tag="ost", bufs=2, name="ost")
            nc.vector.tensor_copy(ost[:], p[:])
            nc.sync.dma_start(
                out=out_d[s * P:(s + 1) * P, nt * N:(nt + 1) * N], in_=ost[:])

        def att_head(qi, h):
            t, hb = h // 2, (h % 2) * HD
            nk = 4 * qi + 4
            ctx_acc = ps.tile([HD + 1, N], F32, tag="ctx", bufs=2,
                              name="ctx_acc")

            def scores_pair(pb):
                # two key tiles share one 2-bank PSUM tile; full pairs get one
                # merged exp, diagonal tiles column-restricted exps + tri mask
                sc = ps.tile([P, 2, N], F32, tag="sc", bufs=2, name="sc")
                ex = sb.tile([P, 2, N], F32R, tag="ex", bufs=2, name="ex")
                n0s = []
                for i in range(2):
                    ki = 2 * pb + i
                    off = ki - 4 * qi
                    n0 = max(0, off) * P
                    n0s.append(n0)
                    nc.tensor.matmul(
                        sc[:, i, n0:],
                        kT[t][hb:hb + HD, ki * P:(ki + 1) * P],
                        qTq[qi][t][hb:hb + HD, n0:],
                        start=True, stop=True,
                    )
                if n0s[0] == 0 and n0s[1] == 0 and 2 * pb + 1 < 4 * qi:
                    nc.scalar.activation(
                        ex.rearrange("p a b -> p (a b)"),
                        sc.rearrange("p a b -> p (a b)"),
                        EXP, scale=0.125)
                else:
                    for i in range(2):
                        n0 = n0s[i]
                        nc.scalar.activation(
                            ex[:, i, n0:], sc[:, i, n0:], EXP, scale=0.125)
                        if 2 * pb + i - 4 * qi >= 0:
                            nc.vector.tensor_mul(
                                ex[:, i, n0:n0 + P], ex[:, i, n0:n0 + P],
                                tri[:])
                return ex, n0s

            def pv_pair(pb, ex, n0s):
                for i in range(2):
                    ki = 2 * pb + i
                    n0 = n0s[i]
                    nc.tensor.matmul(
                        ctx_acc[:, n0:], v_aug[ki][:, h, :], ex[:, i, n0:],
                        start=(ki == 0), stop=(ki == nk - 1),
                    )

            npairs = nk // 2
            exs = []
            for pb in range(npairs):
                exs.append(scores_pair(pb))
                if pb >= 1:
                    pv_pair(pb - 1, *exs[pb - 1])
                    exs[pb - 1] = None
                if pb == 1:
                    finalize()      # previous head, masked by 6 PE MMs
                    emit_op_group()
            pv_pair(npairs - 1, *exs[npairs - 1])
            pending_fin.append((qi, h, ctx_acc))
            if nk == 4:
                emit_op_group()

        def queue_outproj(qi):
            for s4 in range(4):
                for nt in range(2):
                    pending_op.append(
                        lambda qi=qi, s4=s4, nt=nt: outproj_group(qi, s4, nt))

        # ---- interleaved emission ----
        # window 0: phase1(0) + att(0); window w: att(w) + phase1(w+1) +
        # outproj(w-1) sprinkles
        for qi in range(NQT):
            ctxq[qi] = [sb.tile([P, N], BF16, tag=f"ctx{t}", bufs=2,
                                name=f"ctx{t}_{qi}") for t in range(4)]

        def phase1_groups(qi):
            return ([lambda j=j, q=qi: v_group(q, j) for j in range(4)]
                    + [lambda d=d, q=qi: q_group(q, d) for d in range(4)]
                    + [lambda d=d, q=qi: k_group(q, d) for d in range(4)])

        # prologue: V(0), Q(0); K(0) is interleaved with the first heads
        for j in range(4):
            v_group(0, j)
        for dt in range(4):
            q_group(0, dt)

        for qi in range(NQT):
            ph = []
            if qi == 0:
                ph += [lambda t=t: k_group(0, t) for t in range(4)]
            if qi + 1 < NQT:
                ph += phase1_groups(qi + 1)
            npg = len(ph)
            for h in range(HG):
                for _ in range((npg * (h + 1)) // HG - (npg * h) // HG):
                    ph.pop(0)()
                att_head(qi, h)
            finalize()
            queue_outproj(qi)
        while pending_op:
            emit_op_group()

    nc.compile()
    return nc


def _get_nc():
    if "nc" not in _cached:
        _cached["nc"] = _build()
    return _cached["nc"]


def kernel(x, Wq, Wk, Wv, Wo, bo):
    x = np.asarray(x, dtype=np.float32)
    Wq = np.asarray(Wq, dtype=np.float32)
    Wk = np.asarray(Wk, dtype=np.float32)
    Wv = np.asarray(Wv, dtype=np.float32)
    Wo = np.asarray(Wo, dtype=np.float32)
    bo = np.asarray(bo, dtype=np.float32)

    nc = _get_nc()
    in_maps = []
    for c in range(8):
        b, hg = c // 2, c % 2
        cs = slice(hg * DG, (hg + 1) * DG)
        in_maps.append({
            "xT": np.ascontiguousarray(x[b].T),
            "wq": np.ascontiguousarray(Wq[:, cs]),
            "wk": np.ascontiguousarray(Wk[:, cs]),
            "wv": np.ascontiguousarray(Wv[:, cs]),
            "wo": np.ascontiguousarray(Wo[cs, :]).astype(ml_dtypes.bfloat16),
        })
    res = run_bass_kernel_spmd(nc, in_maps, list(range(8)))
    out = np.empty((B, S, DM), np.float32)
    for b in range(B):
        out[b] = res.results[2 * b]["out"] + res.results[2 * b + 1]["out"] + bo
    return out


if __name__ == "__main__":
    rng = np.random.default_rng(0)
    ins = {
        "x": rng.standard_normal((B, S, DM), dtype=np.float32),
        "Wq": rng.standard_normal((DM, DM), dtype=np.float32) / 32,
        "Wk": rng.standard_normal((DM, DM), dtype=np.float32) / 32,
        "Wv": rng.standard_normal((DM, DM), dtype=np.float32) / 32,
        "Wo": rng.standard_normal((DM, DM), dtype=np.float32) / 32,
        "bo": rng.standard_normal((DM,), dtype=np.float32) * 0.01,
    }
    out = kernel(**ins)
    print("kernel ran, out shape", out.shape, "mean", float(np.abs(out).mean()))
